# revision 1
# baseline (speedup 1.0000x reference)
"""DSAttention Trainium2 kernel (8 NeuronCores, SPMD).

Sharding: batch (B=2) x head-groups (4 heads each) -> 8 cores.
Core c handles batch b=c//4, heads 4*(c%4) .. 4*(c%4)+3.

Per-core math (feature-major "transposed" layouts so softmax bias/scale land
on partition axes):
  q_t = Wq_p @ hs_b.T          [256, 2048]   (+bq per-partition)
  k_t = Wk_p @ hs_b.T          [256, 2048]   (+bk per-partition)
  v   = hs_b @ Wv_p.T          [2048, 256]   (per k-tile, with a ones column
                                              per head -> softmax denominator)
  s_t[k, q] = k_t.T q_t        per head, one k-tile x all 2048 q at a time
  e_t = exp(s_t * tau/8 + delta_k/8)         (fused ACT exp, N=1024 halves;
                                              no max-subtraction: |logits|<~12)
  ctx_t[65, q] = [v | 1].T @ e_t             accumulated over 16 k-tiles;
                                              row 64 = denominator
  ctx_t[0:64] *= 1/ctx_t[64]                 (PE rank-1 broadcast of d, then
                                              64-lane DVE reciprocal + mul)
  out_partial = ctx.T @ Wo_p.T               [2048, 1024]
Host: out[b] = sum of the 4 head-group partials + bv @ Wo.T + bo
(softmax rows sum to 1, so the v/out biases commute to the host exactly).

All matmuls in float32r (~1.2e-4 input rounding, full PE rate at N>=256).
Phase B is software-pipelined: ctx matmuls for k-tile kt-1 are emitted after
the scores matmuls for kt so the PE queue never drains waiting on ACT.
"""

import sys

for _p in ("/opt/trn_rl_repo", "/opt/pypackages"):
    if _p not in sys.path:
        sys.path.append(_p)

import numpy as np

import concourse.bass as bass
import concourse.tile as tile
from concourse import bacc, mybir
from concourse.bass_utils import run_bass_kernel_spmd

B, L, H = 2, 2048, 1024
NH, HD = 16, 64
NCORES = 8
HPC = 4  # heads per core
FPC = HPC * HD  # 256
NKT = L // 128  # 16 k-tiles
NHC = H // 128  # 8 H-contraction chunks

F32 = mybir.dt.float32
F32R = mybir.dt.float32r

_NC_CACHE = {}

# Dedup consecutive identical LDWEIGHTS in walrus codegen: every fp32r matmul
# self-loads its stationary operand, and consecutive matmuls often share it.
import concourse.bass_utils as _bu

_orig_run_command = _bu.run_command


def _run_command_ldwopt(cmd, *a, **kw):
    if isinstance(cmd, list):
        cmd = [
            "--enable-ldw-opt=true" if c == "--enable-ldw-opt=false" else c
            for c in cmd
        ]
    return _orig_run_command(cmd, *a, **kw)


_bu.run_command = _run_command_ldwopt


def _build_kernel():
    nc = bacc.Bacc(None, target_bir_lowering=False, debug=False)

    hs_t = nc.declare_dram_parameter("hs_t", [H, L], F32, isOutput=False)
    wq_t = nc.declare_dram_parameter("wq_t", [H, FPC], F32, isOutput=False)
    wk_t = nc.declare_dram_parameter("wk_t", [H, FPC], F32, isOutput=False)
    wv_t = nc.declare_dram_parameter("wv_t", [H, FPC], F32, isOutput=False)
    wo_t = nc.declare_dram_parameter("wo_t", [FPC, H], F32, isOutput=False)
    bq2 = nc.declare_dram_parameter("bq2", [128, 2], F32, isOutput=False)
    bk2 = nc.declare_dram_parameter("bk2", [128, 2], F32, isOutput=False)
    tau8 = nc.declare_dram_parameter("tau8", [128, 1], F32, isOutput=False)
    delta8 = nc.declare_dram_parameter("delta8", [128, NKT], F32, isOutput=False)
    out = nc.declare_dram_parameter("out", [L, H], F32, isOutput=True)
    scratch = nc.declare_dram_parameter("scratch", [128, 512], F32, isOutput=True)

    with tile.TileContext(nc) as tc:
        with (
            tc.tile_pool(name="persist", bufs=1) as persist,
            tc.tile_pool(name="hsw", bufs=1) as hsw,
            # PSUM: "sc" 2 x [128,1024] slots (4 banks) + "ctx" 4 x 2KB (4 banks)
            tc.tile_pool(name="sc_ps", bufs=2, space="PSUM") as sc_ps,
            tc.tile_pool(name="ctx_ps", bufs=4, space="PSUM") as ctx_ps,
            tc.tile_pool(name="work", bufs=4) as work,
            tc.tile_pool(name="dscratch", bufs=2, space="DRAM") as dscratch,
        ):
            # ---- input loads -------------------------------------------------
            hs_sb = []
            for c in range(NHC):
                t = hsw.tile([128, L], F32R, tag=f"hs{c}", name=f"hs{c}")
                nc.sync.dma_start(out=t[:], in_=hs_t[c * 128 : (c + 1) * 128, :].bitcast(F32R))
                hs_sb.append(t)
            w_sb = {}
            for name, w in (("q", wq_t), ("k", wk_t), ("v", wv_t)):
                tiles = []
                for c in range(NHC):
                    t = hsw.tile([128, FPC], F32R, tag=f"w{name}{c}", name=f"w{name}{c}")
                    nc.scalar.dma_start(out=t[:], in_=w[c * 128 : (c + 1) * 128, :].bitcast(F32R))
                    tiles.append(t)
                w_sb[name] = tiles
            wo_sb = []
            for c in range(2):
                t = persist.tile([128, H], F32R, tag=f"wo{c}", name=f"wo{c}")
                nc.scalar.dma_start(out=t[:], in_=wo_t[c * 128 : (c + 1) * 128, :].bitcast(F32R))
                wo_sb.append(t)
            bq_sb = persist.tile([128, 2], F32, tag="bq")
            nc.sync.dma_start(out=bq_sb[:], in_=bq2[:])
            bk_sb = persist.tile([128, 2], F32, tag="bk")
            nc.sync.dma_start(out=bk_sb[:], in_=bk2[:])
            tau_sb = persist.tile([128, 1], F32, tag="tau")
            nc.sync.dma_start(out=tau_sb[:], in_=tau8[:])
            del8_sb = persist.tile([128, NKT], F32, tag="del8")
            nc.sync.dma_start(out=del8_sb[:], in_=delta8[:])
            vones_f = persist.tile([128, HPC], F32, tag="vones_f")
            nc.vector.memset(vones_f[:], 1.0)

            # ---- phase A: projections ---------------------------------------
            q_sb = [persist.tile([128, L], F32R, tag=f"q{hp}", name=f"q{hp}") for hp in range(2)]
            k_sb = [persist.tile([128, L], F32R, tag=f"k{hp}", name=f"k{hp}") for hp in range(2)]
            for dst, wname, bias in ((q_sb, "q", bq_sb), (k_sb, "k", bk_sb)):
                for hp in range(2):
                    ps2 = [
                        sc_ps.tile([128, 1024], F32, tag="sc", name=f"ps_proj{half}")
                        for half in range(2)
                    ]
                    for c in range(NHC):
                        # one stationary load serves all 4 N=512 matmuls
                        for half in range(2):
                            for s2 in range(2):
                                nc.tensor.matmul(
                                    ps2[half][:, s2 * 512 : (s2 + 1) * 512],
                                    w_sb[wname][c][:, hp * 128 : (hp + 1) * 128],
                                    hs_sb[c][:, half * 1024 + s2 * 512 : half * 1024 + (s2 + 1) * 512],
                                    start=(c == 0),
                                    stop=(c == NHC - 1),
                                )
                    for half in range(2):
                        nc.vector.tensor_scalar_add(
                            dst[hp][:, half * 1024 : half * 1024 + 1024],
                            ps2[half][:],
                            bias[:, hp : hp + 1],
                        )

            # v: per k-tile [128, 4*65]; head h cols h*65..h*65+63, col h*65+64 = 1
            v_sb = [persist.tile([128, HPC * 65], F32R, tag=f"v{kt}", name=f"v{kt}") for kt in range(NKT)]
            for kt in range(NKT):
                ps = ctx_ps.tile([128, FPC], F32, tag="ctx", name="ps_vproj")
                for c in range(NHC):
                    nc.tensor.matmul(
                        ps[:],
                        hs_sb[c][:, kt * 128 : (kt + 1) * 128],
                        w_sb["v"][c][:],
                        start=(c == 0),
                        stop=(c == NHC - 1),
                    )
                v_view = v_sb[kt][:].rearrange("p (h w) -> p h w", h=HPC)
                nc.vector.tensor_copy(
                    v_view[:, :, 0:HD],
                    ps[:].rearrange("p (h w) -> p h w", h=HPC),
                )
                nc.vector.tensor_copy(v_view[:, :, HD : HD + 1].squeeze(), vones_f[:])

            # ---- phase B (+ phase C interleaved) ----------------------------
            # Structure: head x q-half x k-tile. Per k-tile: 2 scores matmuls
            # (N=512), 1 fused exp (N=1024), 2 ctx matmuls, and 1 K=128
            # "keepalive" filler matmul. The filler keeps the PE's HAM activity
            # monitor warm: K=64 scores matmuls alone do not register as busy,
            # so one throttle event would otherwise pin the phase at 1.2 GHz.
            # PSUM "ctx" tag rotation (4 slots): 2 ctx accumulators + 1 filler
            # + 1 spare used by the interleaved output-projection chunks.
            ctx_sb = [persist.tile([128, L], F32R, tag=f"ctx{hp}", name=f"ctx{hp}") for hp in range(2)]
            last_fill = [None]

            def emit_c_chunk(lts, paired=False):
                for lt in lts:
                    if paired:
                        # 2 psum tiles, one LDW per c-chunk serving both nch
                        pso = [
                            ctx_ps.tile([128, 512], F32, tag="ctx", name=f"ps_o{n}")
                            for n in range(2)
                        ]
                        for c in range(2):
                            for nch in range(2):
                                nc.tensor.matmul(
                                    pso[nch][:],
                                    ctx_sb[c][:, lt * 128 : (lt + 1) * 128],
                                    wo_sb[c][:, nch * 512 : (nch + 1) * 512],
                                    start=(c == 0),
                                    stop=(c == 1),
                                )
                        for nch in range(2):
                            o_sb = work.tile([128, 512], F32, tag="ostage", name="o_sb", bufs=3)
                            nc.vector.tensor_copy(o_sb[:], pso[nch][:])
                            nc.sync.dma_start(
                                out=out[lt * 128 : (lt + 1) * 128, nch * 512 : (nch + 1) * 512],
                                in_=o_sb[:],
                            )
                        continue
                    # serial PSUM use: one pso tile in flight at a time
                    for nch in range(2):
                        pso = ctx_ps.tile([128, 512], F32, tag="ctx", name="ps_o")
                        for c in range(2):
                            nc.tensor.matmul(
                                pso[:],
                                ctx_sb[c][:, lt * 128 : (lt + 1) * 128],
                                wo_sb[c][:, nch * 512 : (nch + 1) * 512],
                                start=(c == 0),
                                stop=(c == 1),
                            )
                        o_sb = work.tile([128, 512], F32, tag="ostage", name="o_sb", bufs=3)
                        nc.vector.tensor_copy(o_sb[:], pso[:])
                        nc.sync.dma_start(
                            out=out[lt * 128 : (lt + 1) * 128, nch * 512 : (nch + 1) * 512],
                            in_=o_sb[:],
                        )

            for h in range(HPC):
                hp, hr = divmod(h, 2)
                q_head = q_sb[hp][hr * HD : (hr + 1) * HD, :]
                k_head = k_sb[hp][hr * HD : (hr + 1) * HD, :]
                for half in range(2):
                    qoff = half * 1024
                    ctx2 = [
                        ctx_ps.tile(
                            [65, 512], F32, tag="ctx", name=f"ctx_h{h}f{half}{g2}"
                        )
                        for g2 in range(2)
                    ]
                    fill_ps = ctx_ps.tile([65, 512], F32, tag="ctx", name="fill_ps")
                    last_fill[0] = fill_ps
                    nfill = [0]

                    def emit_filler(kt0, h=h, fill_ps=fill_ps, nfill=nfill):
                        # K=128 keepalive reusing the ctx pair's stationary
                        nc.tensor.matmul(
                            fill_ps[:],
                            v_sb[kt0][:, h * 65 : (h + 1) * 65],
                            hs_sb[0][:, 0:512].bitcast(F32R),
                            start=(nfill[0] == 0),
                            stop=(nfill[0] == NKT - 2),
                            skip_group_check=True,
                        )
                        nfill[0] += 1

                    prev = None  # (kt, e)

                    def emit_ctx(prev, h=h, ctx2=ctx2):
                        kt0, e = prev
                        for g2 in range(2):
                            nc.tensor.matmul(
                                ctx2[g2][:],
                                v_sb[kt0][:, h * 65 : (h + 1) * 65],
                                e[:, g2 * 512 : (g2 + 1) * 512],
                                start=(kt0 == 0),
                                stop=(kt0 == NKT - 1),
                            )

                    for kt in range(NKT):
                        psS = sc_ps.tile([128, 1024], F32, tag="sc", name="ps_s")
                        for s2 in range(2):
                            nc.tensor.matmul(
                                psS[:, s2 * 512 : (s2 + 1) * 512],
                                k_head[:, kt * 128 : (kt + 1) * 128],
                                q_head[:, qoff + s2 * 512 : qoff + (s2 + 1) * 512],
                                start=True,
                                stop=True,
                            )
                        if h == HPC - 1 and half == 1 and kt == 6:
                            emit_c_chunk(range(0, 8))
                        if prev is not None:
                            emit_ctx(prev)
                            emit_filler(prev[0])
                        e_t = work.tile([128, 1024], F32R, tag="e", name="e_t", bufs=3)
                        nc.scalar.activation(
                            e_t[:],
                            psS[:],
                            mybir.ActivationFunctionType.Exp,
                            bias=del8_sb[:, kt : kt + 1],
                            scale=tau_sb[:],
                        )
                        prev = (kt, e_t)
                    emit_ctx(prev)

                    # normalize ctx[0:64] / ctx[64]: drain PSUM -> SBUF at once
                    # (frees accumulator banks), then broadcast the denominator
                    # row via DRAM-bounce DMA and divide on DVE — no PE/PSUM.
                    raws = []
                    for g2 in range(2):
                        raw = work.tile([65, 512], F32R, tag="raw", name=f"raw{g2}", bufs=2)
                        nc.vector.tensor_copy(raw[:], ctx2[g2][:])
                        raws.append(raw)
                    for g2 in range(2):
                        g_abs = half * 2 + g2
                        d_dram = dscratch.tile([1, 512], F32, tag="ddram", name="d_dram")
                        nc.sync.dma_start(out=d_dram[:], in_=raws[g2][64:65, :].bitcast(F32))
                        d_bc = work.tile([64, 512], F32, tag="dbc", name="d_bc", bufs=2)
                        nc.sync.dma_start(
                            out=d_bc[:],
                            in_=d_dram[0:1, :].to_broadcast([64, 512]),
                        )
                        r_sb = work.tile([64, 512], F32, tag="r", name="r_sb", bufs=2)
                        nc.vector.reciprocal(r_sb[:], d_bc[:])
                        nc.vector.tensor_mul(
                            ctx_sb[hp][hr * HD : (hr + 1) * HD, g_abs * 512 : (g_abs + 1) * 512],
                            raws[g2][0:64, :],
                            r_sb[:],
                        )
            emit_c_chunk(range(8, 16), paired=True)

            # read the last filler accumulator so DCE keeps the keepalives
            fcopy = work.tile([65, 512], F32, tag="ostage", name="fcopy", bufs=3)
            nc.vector.tensor_copy(fcopy[:], last_fill[0][:])
            nc.sync.dma_start(out=scratch[0:65, :], in_=fcopy[:])

    nc.compile()
    return nc


def _get_nc():
    if "nc" not in _NC_CACHE:
        _NC_CACHE["nc"] = _build_kernel()
    return _NC_CACHE["nc"]


def _make_in_maps(hidden_states, tau, delta, Wq, Wk, Wv, Wo, bq, bk):
    in_maps = []
    for c in range(NCORES):
        b, hg = divmod(c, HPC)
        fs = slice(hg * FPC, (hg + 1) * FPC)
        in_maps.append(
            {
                "hs_t": np.ascontiguousarray(hidden_states[b].T),
                "wq_t": np.ascontiguousarray(Wq[fs, :].T),
                "wk_t": np.ascontiguousarray(Wk[fs, :].T),
                "wv_t": np.ascontiguousarray(Wv[fs, :].T),
                "wo_t": np.ascontiguousarray(Wo[:, fs].T),
                "bq2": np.ascontiguousarray(bq[fs].reshape(2, 128).T),
                "bk2": np.ascontiguousarray(bk[fs].reshape(2, 128).T),
                "tau8": np.full((128, 1), tau[b, 0] / 8.0, dtype=np.float32),
                "delta8": np.ascontiguousarray((delta[b] / 8.0).reshape(NKT, 128).T),
            }
        )
    return in_maps


def kernel(hidden_states, tau, delta, Wq, bq, Wk, bk, Wv, bv, Wo, bo, _trace=False):
    hidden_states = np.asarray(hidden_states, dtype=np.float32)
    tau = np.asarray(tau, dtype=np.float32)
    delta = np.asarray(delta, dtype=np.float32)
    Wq = np.asarray(Wq, dtype=np.float32)
    Wk = np.asarray(Wk, dtype=np.float32)
    Wv = np.asarray(Wv, dtype=np.float32)
    Wo = np.asarray(Wo, dtype=np.float32)
    bq = np.asarray(bq, dtype=np.float32)
    bk = np.asarray(bk, dtype=np.float32)
    bv = np.asarray(bv, dtype=np.float32)
    bo = np.asarray(bo, dtype=np.float32)

    nc = _get_nc()
    in_maps = _make_in_maps(hidden_states, tau, delta, Wq, Wk, Wv, Wo, bq, bk)
    res = run_bass_kernel_spmd(nc, in_maps, list(range(NCORES)), trace=_trace)

    out = np.zeros((B, L, H), dtype=np.float32)
    for c in range(NCORES):
        out[c // HPC] += res.results[c]["out"]
    # v/out-proj biases commute through softmax-normalized attention exactly
    out += bv @ Wo.T + bo
    if _trace:
        kernel._last_exec_time_ns = res.exec_time_ns
        kernel._last_profile_json = res.profile_json
    return out



# revision 10
# speedup vs baseline: 1.2310x; 1.2310x over previous
"""DSAttention Trainium2 kernel (8 NeuronCores, SPMD).

Sharding: batch (B=2) x head-groups (4 heads each) -> 8 cores.
Core c handles batch b=c//4, heads 4*(c%4) .. 4*(c%4)+3.

The kernel is ACT(exp)-bound: 128 exp tiles of [128, 1024] at ~1.2us each
(~154us) is the floor. Everything else (projections, scores, ctx, output
projection, DMA) is arranged to hide under that ACT stream:

  pre-B:  q/k for heads 0-1 accumulate per hs-chunk as DMA lands, then the
          16 v k-tiles (hs-stationary matmuls). First exp at ~28us.
  phase B (per head h, per q-half): 16 k-tiles; per k-tile 2 scores MMs
          (K=64) -> fused exp (scale tau/8, bias delta_k/8, bf16 out) ->
          2 ctx MMs (K=128, [v|1] stationary, row 64 = denominator).
          PE slack per tile (~340ns) absorbs "side work" closures:
          q/k-half1 + heads-2-3 projections (units 0-3), keepalive filler
          (units 4-6), first-half output projection (unit 7).
  normalize per (h, half): PSUM drain -> reciprocal_approx_fast on the
          denominator row -> DRAM-bounce broadcast -> DVE multiply.
  tail:   second-half output projection + bf16 output DMA.

All matmuls and SBUF operands bf16 (PSUM accumulation fp32; host-simulated
rel err 5.6e-3 vs the 2e-2 gate). PSUM: sc 2x[128,1024] (4 banks) +
ctx 2x[65,512] (2) + floater 2x[128,512] (2) = 8 banks exactly.
Host: out[b] = sum of the 4 head-group partials + bv @ Wo.T + bo
(softmax rows sum to 1, so the v/out biases commute to the host exactly).
"""

import sys

for _p in ("/opt/trn_rl_repo", "/opt/pypackages"):
    if _p not in sys.path:
        sys.path.append(_p)

import numpy as np
import ml_dtypes

import concourse.bass as bass
import concourse.tile as tile
from concourse import bacc, mybir
from concourse.bass_utils import run_bass_kernel_spmd

B, L, H = 2, 2048, 1024
NH, HD = 16, 64
NCORES = 8
HPC = 4  # heads per core
FPC = HPC * HD  # 256
NKT = L // 128  # 16 k-tiles
NHC = H // 128  # 8 H-contraction chunks

F32 = mybir.dt.float32
BF16 = mybir.dt.bfloat16
NPBF = ml_dtypes.bfloat16

_NC_CACHE = {}


def _build_kernel():
    nc = bacc.Bacc(None, target_bir_lowering=False, debug=False)

    hs_t = nc.declare_dram_parameter("hs_t", [H, L], BF16, isOutput=False)
    wq0 = nc.declare_dram_parameter("wq0", [H, 128], BF16, isOutput=False)
    wk0 = nc.declare_dram_parameter("wk0", [H, 128], BF16, isOutput=False)
    wq1 = nc.declare_dram_parameter("wq1", [H, 128], BF16, isOutput=False)
    wk1 = nc.declare_dram_parameter("wk1", [H, 128], BF16, isOutput=False)
    wv_t = nc.declare_dram_parameter("wv_t", [H, FPC], BF16, isOutput=False)
    wo_t = nc.declare_dram_parameter("wo_t", [FPC, H], BF16, isOutput=False)
    bq2 = nc.declare_dram_parameter("bq2", [128, 2], F32, isOutput=False)
    bk2 = nc.declare_dram_parameter("bk2", [128, 2], F32, isOutput=False)
    tau8 = nc.declare_dram_parameter("tau8", [128, 1], F32, isOutput=False)
    delta8 = nc.declare_dram_parameter("delta8", [128, NKT], F32, isOutput=False)
    out = nc.declare_dram_parameter("out", [L, H], BF16, isOutput=True)
    scratch = nc.declare_dram_parameter("scratch", [128, 512], F32, isOutput=True)

    with tile.TileContext(nc) as tc:
        with (
            tc.tile_pool(name="persist", bufs=1) as persist,
            tc.tile_pool(name="hsw", bufs=1) as hsw,
            # PSUM (8 banks): sc 2x[128,1024] + cg 2x[65,512] + fl 2x[128,512]
            tc.tile_pool(name="sc_ps", bufs=2, space="PSUM") as sc_ps,
            tc.tile_pool(name="cg_ps", bufs=2, space="PSUM") as cg_ps,
            tc.tile_pool(name="fl_ps", bufs=2, space="PSUM") as fl_ps,
            tc.tile_pool(name="work", bufs=4) as work,
            tc.tile_pool(name="dscratch", bufs=2, space="DRAM") as dscratch,
        ):
            # ---- input DMAs ------------------------------------------------
            hs_sb = []
            for c in range(NHC):
                t = hsw.tile([128, L], BF16, tag=f"hs{c}", name=f"hs{c}")
                nc.sync.dma_start(out=t[:], in_=hs_t[c * 128 : (c + 1) * 128, :])
                hs_sb.append(t)
            bq_sb = persist.tile([128, 2], F32, tag="bq")
            nc.scalar.dma_start(out=bq_sb[:], in_=bq2[:])
            bk_sb = persist.tile([128, 2], F32, tag="bk")
            nc.scalar.dma_start(out=bk_sb[:], in_=bk2[:])
            tau_sb = persist.tile([128, 1], F32, tag="tau")
            nc.scalar.dma_start(out=tau_sb[:], in_=tau8[:])
            del8_sb = persist.tile([128, NKT], F32, tag="del8")
            nc.scalar.dma_start(out=del8_sb[:], in_=delta8[:])
            # weights on the scalar queue (FIFO: all land before the first
            # exp issues), ordered by when pre-B needs them
            w_sb = {}
            for name, w in (
                ("q0", wq0),
                ("k0", wk0),
                ("v", wv_t),
                ("q1", wq1),
                ("k1", wk1),
            ):
                tiles = []
                ncol = w.shape[1]
                for c in range(NHC):
                    t = hsw.tile([128, ncol], BF16, tag=f"w{name}{c}", name=f"w{name}{c}")
                    nc.scalar.dma_start(out=t[:], in_=w[c * 128 : (c + 1) * 128, :])
                    tiles.append(t)
                w_sb[name] = tiles
            wo_sb = []
            for c in range(2):
                t = persist.tile([128, H], BF16, tag=f"wo{c}", name=f"wo{c}")
                nc.scalar.dma_start(out=t[:], in_=wo_t[c * 128 : (c + 1) * 128, :])
                wo_sb.append(t)
            vones_f = persist.tile([128, HPC], BF16, tag="vones_f")
            nc.vector.memset(vones_f[:], 1.0)

            q_sb = [persist.tile([128, L], BF16, tag=f"q{hp}", name=f"q{hp}") for hp in range(2)]
            k_sb = [persist.tile([128, L], BF16, tag=f"k{hp}", name=f"k{hp}") for hp in range(2)]
            ctx_sb = [persist.tile([128, L], BF16, tag=f"ctx{hp}", name=f"ctx{hp}") for hp in range(2)]
            v_sb = [persist.tile([128, HPC * 65], BF16, tag=f"v{kt}", name=f"v{kt}") for kt in range(NKT)]

            # ---- pre-B: q/k hp0 first halves, DMA-paced per chunk ----------
            # (scores scan k over the FULL key length every unit, so k-hp0
            # must be complete before phase B; q stays lazy per half)
            psA = sc_ps.tile([128, 1024], F32, tag="sc", name="ps_q0a")
            psB = sc_ps.tile([128, 1024], F32, tag="sc", name="ps_k0a")
            for c in range(NHC):
                for s2 in range(2):
                    nc.tensor.matmul(
                        psA[:, s2 * 512 : (s2 + 1) * 512],
                        w_sb["q0"][c][:],
                        hs_sb[c][:, s2 * 512 : (s2 + 1) * 512],
                        start=(c == 0),
                        stop=(c == NHC - 1),
                    )
                for s2 in range(2):
                    nc.tensor.matmul(
                        psB[:, s2 * 512 : (s2 + 1) * 512],
                        w_sb["k0"][c][:],
                        hs_sb[c][:, s2 * 512 : (s2 + 1) * 512],
                        start=(c == 0),
                        stop=(c == NHC - 1),
                    )
            nc.vector.tensor_scalar_add(q_sb[0][:, 0:1024], psA[:], bq_sb[:, 0:1])
            nc.vector.tensor_scalar_add(k_sb[0][:, 0:1024], psB[:], bk_sb[:, 0:1])

            # ---- pre-B: v projection (all k-tiles) + k-hp0 second half -----
            psK = sc_ps.tile([128, 1024], F32, tag="sc", name="ps_k0b")
            for kt in range(NKT):
                pv = fl_ps.tile([128, 512], F32, tag="fl", name="ps_v")
                for c in range(NHC):
                    nc.tensor.matmul(
                        pv[:, 0:FPC],
                        hs_sb[c][:, kt * 128 : (kt + 1) * 128],
                        w_sb["v"][c][:],
                        start=(c == 0),
                        stop=(c == NHC - 1),
                    )
                if kt < NHC:
                    c = kt
                    for s2 in range(2):
                        nc.tensor.matmul(
                            psK[:, s2 * 512 : (s2 + 1) * 512],
                            w_sb["k0"][c][:],
                            hs_sb[c][:, 1024 + s2 * 512 : 1024 + (s2 + 1) * 512],
                            start=(c == 0),
                            stop=(c == NHC - 1),
                        )
                v_view = v_sb[kt][:].rearrange("p (h w) -> p h w", h=HPC)
                nc.vector.tensor_copy(
                    v_view[:, :, 0:HD],
                    pv[:, 0:FPC].rearrange("p (h w) -> p h w", h=HPC),
                )
                nc.vector.tensor_copy(v_view[:, :, HD : HD + 1].squeeze(), vones_f[:])
            nc.vector.tensor_scalar_add(k_sb[0][:, 1024:2048], psK[:], bk_sb[:, 0:1])

            # ---- side-work closures (emitted into phase-B PE slack) --------
            # Each closure emits one PE matmul (or one DVE drain). Groups:
            # q/k hp0 second halves, q/k hp1 both halves -> 12 accumulation
            # groups of [128, 512] in the floater psum + a drain each.
            # qk_q drains 2/iter in units 0-3; barriers before units that
            # read the projected regions flush any stragglers.
            qk_q = []
            op_q = []

            def make_qk_group(wname, colofs, dst, bias, bcol):
                state = {}

                def mk(c, colofs=colofs, wname=wname, state=state):
                    def emit():
                        if c == 0:
                            state["ps"] = fl_ps.tile(
                                [128, 512], F32, tag="fl", name=f"ps_{wname}{colofs}"
                            )
                        nc.tensor.matmul(
                            state["ps"][:],
                            w_sb[wname][c][:],
                            hs_sb[c][:, colofs : colofs + 512],
                            start=(c == 0),
                            stop=(c == NHC - 1),
                        )

                    return emit

                for c in range(NHC):
                    qk_q.append(mk(c))

                def drain(dst=dst, colofs=colofs, bias=bias, bcol=bcol, state=state):
                    nc.vector.tensor_scalar_add(
                        dst[:, colofs : colofs + 512],
                        state["ps"][:],
                        bias[:, bcol : bcol + 1],
                    )

                qk_q.append(drain)

            for colofs in (1024, 1536):
                make_qk_group("q0", colofs, q_sb[0], bq_sb, 0)
            n_qk_half1 = len(qk_q)  # unit 1 (h0, half1) needs these done
            for colofs in (0, 512, 1024, 1536):
                make_qk_group("k1", colofs, k_sb[1], bk_sb, 1)
                make_qk_group("q1", colofs, q_sb[1], bq_sb, 1)
            n_qk_total = len(qk_q)
            qk_done = [0]

            def pop_qk(n):
                while n > 0 and qk_q:
                    qk_q.pop(0)()
                    qk_done[0] += 1
                    n -= 1

            def flush_qk_until(target):
                pop_qk(target - qk_done[0])

            # Output projection closures for one q-half: 16 groups of
            # (2 accum MMs -> drain -> DMA out).
            def push_oproj_half(half):
                for lt in range(half * 8, half * 8 + 8):
                    for nch in range(2):
                        state = {}

                        def mm(cc, lt=lt, nch=nch, state=state):
                            def emit():
                                if cc == 0:
                                    state["ps"] = fl_ps.tile(
                                        [128, 512], F32, tag="fl", name="ps_o"
                                    )
                                nc.tensor.matmul(
                                    state["ps"][:],
                                    ctx_sb[cc][:, lt * 128 : (lt + 1) * 128],
                                    wo_sb[cc][:, nch * 512 : (nch + 1) * 512],
                                    start=(cc == 0),
                                    stop=(cc == 1),
                                )

                            return emit

                        op_q.append(mm(0))
                        op_q.append(mm(1))

                        def drain(lt=lt, nch=nch, state=state):
                            o_sb = work.tile(
                                [128, 512], BF16, tag="ostage", name="o_sb", bufs=3
                            )
                            nc.vector.tensor_copy(o_sb[:], state["ps"][:])
                            nc.sync.dma_start(
                                out=out[
                                    lt * 128 : (lt + 1) * 128,
                                    nch * 512 : (nch + 1) * 512,
                                ],
                                in_=o_sb[:],
                            )

                        op_q.append(drain)

            # ---- phase B ---------------------------------------------------
            fill_state = {"ps": None, "n": 0}

            def normalize(h, half, cg):
                hp, hr = divmod(h, 2)
                raw = work.tile([65, 1024], F32, tag="raw", name="raw", bufs=2)
                for g2 in range(2):
                    nc.vector.tensor_copy(raw[:, g2 * 512 : (g2 + 1) * 512], cg[g2][:])
                d_dram = dscratch.tile([1, 1024], F32, tag="ddram", name="d_dram")
                nc.sync.dma_start(out=d_dram[:], in_=raw[64:65, :])
                dbc = work.tile([64, 1024], F32, tag="dbc", name="dbc", bufs=2)
                nc.sync.dma_start(
                    out=dbc[:], in_=d_dram[0:1, :].to_broadcast([64, 1024])
                )
                rbc = work.tile([64, 1024], F32, tag="rbc", name="rbc", bufs=2)
                nc.vector.reciprocal_approx_fast(rbc[:], dbc[:])
                nc.vector.tensor_mul(
                    ctx_sb[hp][hr * HD : (hr + 1) * HD, half * 1024 : (half + 1) * 1024],
                    raw[0:64, :],
                    rbc[:],
                )

            for h in range(HPC):
                hp, hr = divmod(h, 2)
                q_head = q_sb[hp][hr * HD : (hr + 1) * HD, :]
                k_head = k_sb[hp][hr * HD : (hr + 1) * HD, :]
                for half in range(2):
                    unit = h * 2 + half
                    qoff = half * 1024
                    # correctness barriers: the unit's q/k regions must be
                    # fully projected before its scores MMs are emitted
                    if unit == 1:
                        flush_qk_until(n_qk_half1)
                    elif unit == 4:
                        flush_qk_until(n_qk_total)
                    cg = [
                        cg_ps.tile([65, 512], F32, tag="cg", name=f"cg{h}{half}{g2}")
                        for g2 in range(2)
                    ]
                    prev = None

                    def emit_ctx(prev, h=h, cg=cg):
                        kt0, e = prev
                        for g2 in range(2):
                            nc.tensor.matmul(
                                cg[g2][:],
                                v_sb[kt0][:, h * 65 : (h + 1) * 65],
                                e[:, g2 * 512 : (g2 + 1) * 512],
                                start=(kt0 == 0),
                                stop=(kt0 == NKT - 1),
                            )

                    for kt in range(NKT):
                        psS = sc_ps.tile([128, 1024], F32, tag="sc", name="ps_s")
                        for s2 in range(2):
                            nc.tensor.matmul(
                                psS[:, s2 * 512 : (s2 + 1) * 512],
                                k_head[:, kt * 128 : (kt + 1) * 128],
                                q_head[:, qoff + s2 * 512 : qoff + (s2 + 1) * 512],
                                start=True,
                                stop=True,
                            )
                        # side work in the PE slack of the ACT-paced loop
                        if qk_q:
                            pop_qk(2)
                        elif op_q:
                            op_q.pop(0)()
                            if op_q:
                                op_q.pop(0)()
                        elif 2 <= unit <= 6:
                            # keepalive: HAM ignores K=64 scores; without
                            # K=128 side work the PE clock pins at 1.2 GHz
                            if fill_state["ps"] is None:
                                fill_state["ps"] = fl_ps.tile(
                                    [65, 512], F32, tag="fl", name="fill_ps"
                                )
                                fill_state["n"] = 0
                            nc.tensor.matmul(
                                fill_state["ps"][:],
                                v_sb[kt][:, h * 65 : (h + 1) * 65],
                                hs_sb[0][:, 0:512],
                                start=(fill_state["n"] == 0),
                                stop=False,
                                skip_group_check=True,
                            )
                            fill_state["n"] += 1
                        if prev is not None:
                            emit_ctx(prev)
                        e_t = work.tile([128, 1024], BF16, tag="e", name="e_t", bufs=4)
                        nc.scalar.activation(
                            e_t[:],
                            psS[:],
                            mybir.ActivationFunctionType.Exp,
                            bias=del8_sb[:, kt : kt + 1],
                            scale=tau_sb[:],
                        )
                        prev = (kt, e_t)
                    emit_ctx(prev)
                    # close an open filler accumulation before normalize
                    if fill_state["ps"] is not None:
                        nc.tensor.matmul(
                            fill_state["ps"][:],
                            v_sb[0][:, h * 65 : (h + 1) * 65],
                            hs_sb[0][:, 0:512],
                            start=False,
                            stop=True,
                            skip_group_check=True,
                        )
                        fcopy = work.tile([65, 512], F32, tag="fguard", name="fcopy", bufs=2)
                        nc.vector.tensor_copy(fcopy[:], fill_state["ps"][:])
                        nc.sync.dma_start(out=scratch[0:65, :], in_=fcopy[:])
                        fill_state["ps"] = None
                    normalize(h, half, cg)
                    if h == HPC - 1 and half == 0:
                        push_oproj_half(0)

            push_oproj_half(1)
            while op_q:
                op_q.pop(0)()

    nc.compile()
    return nc


def _get_nc():
    if "nc" not in _NC_CACHE:
        _NC_CACHE["nc"] = _build_kernel()
    return _NC_CACHE["nc"]


def _make_in_maps(hidden_states, tau, delta, Wq, Wk, Wv, Wo, bq, bk):
    def bfc(x):
        return np.ascontiguousarray(x).astype(NPBF)

    in_maps = []
    for c in range(NCORES):
        b, hg = divmod(c, HPC)
        fs = slice(hg * FPC, (hg + 1) * FPC)
        wq_t = Wq[fs, :].T  # [H, FPC]
        wk_t = Wk[fs, :].T
        in_maps.append(
            {
                "hs_t": bfc(hidden_states[b].T),
                "wq0": bfc(wq_t[:, 0:128]),
                "wq1": bfc(wq_t[:, 128:256]),
                "wk0": bfc(wk_t[:, 0:128]),
                "wk1": bfc(wk_t[:, 128:256]),
                "wv_t": bfc(Wv[fs, :].T),
                "wo_t": bfc(Wo[:, fs].T),
                "bq2": np.ascontiguousarray(bq[fs].reshape(2, 128).T),
                "bk2": np.ascontiguousarray(bk[fs].reshape(2, 128).T),
                "tau8": np.full((128, 1), tau[b, 0] / 8.0, dtype=np.float32),
                "delta8": np.ascontiguousarray((delta[b] / 8.0).reshape(NKT, 128).T),
            }
        )
    return in_maps


def kernel(hidden_states, tau, delta, Wq, bq, Wk, bk, Wv, bv, Wo, bo, _trace=False):
    hidden_states = np.asarray(hidden_states, dtype=np.float32)
    tau = np.asarray(tau, dtype=np.float32)
    delta = np.asarray(delta, dtype=np.float32)
    Wq = np.asarray(Wq, dtype=np.float32)
    Wk = np.asarray(Wk, dtype=np.float32)
    Wv = np.asarray(Wv, dtype=np.float32)
    Wo = np.asarray(Wo, dtype=np.float32)
    bq = np.asarray(bq, dtype=np.float32)
    bk = np.asarray(bk, dtype=np.float32)
    bv = np.asarray(bv, dtype=np.float32)
    bo = np.asarray(bo, dtype=np.float32)

    nc = _get_nc()
    in_maps = _make_in_maps(hidden_states, tau, delta, Wq, Wk, Wv, Wo, bq, bk)
    res = run_bass_kernel_spmd(nc, in_maps, list(range(NCORES)), trace=_trace)

    out = np.zeros((B, L, H), dtype=np.float32)
    for c in range(NCORES):
        out[c // HPC] += res.results[c]["out"].astype(np.float32)
    # v/out-proj biases commute through softmax-normalized attention exactly
    out += bv @ Wo.T + bo
    if _trace:
        kernel._last_exec_time_ns = res.exec_time_ns
        kernel._last_profile_json = res.profile_json
    return out


# revision 14
# speedup vs baseline: 1.2396x; 1.0070x over previous
"""DSAttention Trainium2 kernel (8 NeuronCores, SPMD).

Sharding: batch (B=2) x head-groups (4 heads each) -> 8 cores.
Core c handles batch b=c//4, heads 4*(c%4) .. 4*(c%4)+3.

The kernel is ACT(exp)-bound: 128 exp tiles of [128, 1024] at ~1.2us each
(~154us) is the floor. Everything else (projections, scores, ctx, output
projection, DMA) is arranged to hide under that ACT stream:

  pre-B:  hs streams in column-groups of 512 so the v k-tile projections
          (hs-stationary matmuls) and q/k head-0/1 projections overlap the
          DMA; first exp at ~35us.
  phase B (per head h, per q-half): 16 k-tiles; per k-tile 2 scores MMs
          (K=64) -> fused exp (scale tau/8, bias delta_k/8, bf16 out) ->
          2 ctx MMs (K=128, [v|1] stationary, row 64 = denominator).
          The last ctx of each unit is carried past the next unit's first
          scores MMs so the ACT stream never waits at unit boundaries.
          PE slack per tile (~340ns) absorbs "side work" closures:
          remaining q/k projections (units 0-3), keepalive filler
          (units 2-6), first-half output projection (unit 7).
  normalize per (h, half): PSUM drain -> denominator row DRAM-bounce
          broadcast -> reciprocal_approx_fast -> DVE multiply.
  tail:   second-half output projection; PSUM drains alternate DVE/ACT
          and output DMAs alternate sync/scalar queues (both idle then).

All matmuls and SBUF operands bf16 (PSUM accumulation fp32; host-simulated
rel err 5.6e-3 vs the 2e-2 gate). PSUM: sc 2x[128,1024] (4 banks) +
ctx 2x[65,512] (2) + floater 2x[128,512] (2) = 8 banks exactly.
Host: out[b] = sum of the 4 head-group partials + bv @ Wo.T + bo
(softmax rows sum to 1, so the v/out biases commute to the host exactly).
"""

import sys

for _p in ("/opt/trn_rl_repo", "/opt/pypackages"):
    if _p not in sys.path:
        sys.path.append(_p)

import numpy as np
import ml_dtypes

import concourse.bass as bass
import concourse.tile as tile
from concourse import bacc, mybir
from concourse.bass_utils import run_bass_kernel_spmd

B, L, H = 2, 2048, 1024
NH, HD = 16, 64
NCORES = 8
HPC = 4  # heads per core
FPC = HPC * HD  # 256
NKT = L // 128  # 16 k-tiles
NHC = H // 128  # 8 H-contraction chunks
NCG = 4  # hs column groups of 512

F32 = mybir.dt.float32
BF16 = mybir.dt.bfloat16
NPBF = ml_dtypes.bfloat16

_NC_CACHE = {}


def _build_kernel():
    nc = bacc.Bacc(None, target_bir_lowering=False, debug=False)

    hs_t = nc.declare_dram_parameter("hs_t", [H, L], BF16, isOutput=False)
    wq0 = nc.declare_dram_parameter("wq0", [H, 128], BF16, isOutput=False)
    wk0 = nc.declare_dram_parameter("wk0", [H, 128], BF16, isOutput=False)
    wq1 = nc.declare_dram_parameter("wq1", [H, 128], BF16, isOutput=False)
    wk1 = nc.declare_dram_parameter("wk1", [H, 128], BF16, isOutput=False)
    wv_t = nc.declare_dram_parameter("wv_t", [H, FPC], BF16, isOutput=False)
    wo_t = nc.declare_dram_parameter("wo_t", [FPC, H], BF16, isOutput=False)
    bq2 = nc.declare_dram_parameter("bq2", [128, 2], F32, isOutput=False)
    bk2 = nc.declare_dram_parameter("bk2", [128, 2], F32, isOutput=False)
    tau8 = nc.declare_dram_parameter("tau8", [128, 1], F32, isOutput=False)
    delta8 = nc.declare_dram_parameter("delta8", [128, NKT], F32, isOutput=False)
    out = nc.declare_dram_parameter("out", [L, H], BF16, isOutput=True)
    scratch = nc.declare_dram_parameter("scratch", [128, 512], F32, isOutput=True)

    with tile.TileContext(nc) as tc:
        with (
            tc.tile_pool(name="persist", bufs=1) as persist,
            tc.tile_pool(name="hsw", bufs=1) as hsw,
            # PSUM (8 banks): sc 2x[128,1024] + cg 2x[65,512] + fl 2x[128,512]
            tc.tile_pool(name="sc_ps", bufs=2, space="PSUM") as sc_ps,
            tc.tile_pool(name="cg_ps", bufs=2, space="PSUM") as cg_ps,
            tc.tile_pool(name="fl_ps", bufs=2, space="PSUM") as fl_ps,
            tc.tile_pool(name="work", bufs=4) as work,
            tc.tile_pool(name="dscratch", bufs=2, space="DRAM") as dscratch,
        ):
            # ---- input DMAs ------------------------------------------------
            # hs column-group-major on sync so v/q/k matmuls start early
            hs_cg = [[None] * NCG for _ in range(NHC)]
            for g in range(NCG):
                for c in range(NHC):
                    t = hsw.tile([128, 512], BF16, tag=f"hs{c}g{g}", name=f"hs{c}g{g}")
                    nc.sync.dma_start(
                        out=t[:],
                        in_=hs_t[c * 128 : (c + 1) * 128, g * 512 : (g + 1) * 512],
                    )
                    hs_cg[c][g] = t
            bq_sb = persist.tile([128, 2], F32, tag="bq")
            nc.scalar.dma_start(out=bq_sb[:], in_=bq2[:])
            bk_sb = persist.tile([128, 2], F32, tag="bk")
            nc.scalar.dma_start(out=bk_sb[:], in_=bk2[:])
            tau_sb = persist.tile([128, 1], F32, tag="tau")
            nc.scalar.dma_start(out=tau_sb[:], in_=tau8[:])
            del8_sb = persist.tile([128, NKT], F32, tag="del8")
            nc.scalar.dma_start(out=del8_sb[:], in_=delta8[:])
            w_sb = {}
            for name, w in (
                ("v", wv_t),
                ("q0", wq0),
                ("k0", wk0),
                ("q1", wq1),
                ("k1", wk1),
            ):
                tiles = []
                ncol = w.shape[1]
                for c in range(NHC):
                    t = hsw.tile([128, ncol], BF16, tag=f"w{name}{c}", name=f"w{name}{c}")
                    nc.scalar.dma_start(out=t[:], in_=w[c * 128 : (c + 1) * 128, :])
                    tiles.append(t)
                w_sb[name] = tiles
            wo_sb = []
            for c in range(2):
                t = persist.tile([128, H], BF16, tag=f"wo{c}", name=f"wo{c}")
                nc.scalar.dma_start(out=t[:], in_=wo_t[c * 128 : (c + 1) * 128, :])
                wo_sb.append(t)
            vones_f = persist.tile([128, HPC], BF16, tag="vones_f")
            nc.vector.memset(vones_f[:], 1.0)

            q_sb = [persist.tile([128, L], BF16, tag=f"q{hp}", name=f"q{hp}") for hp in range(2)]
            k_sb = [persist.tile([128, L], BF16, tag=f"k{hp}", name=f"k{hp}") for hp in range(2)]
            ctx_sb = [persist.tile([128, L], BF16, tag=f"ctx{hp}", name=f"ctx{hp}") for hp in range(2)]
            v_sb = [persist.tile([128, HPC * 65], BF16, tag=f"v{kt}", name=f"v{kt}") for kt in range(NKT)]

            def emit_v_group(kt):
                pv = fl_ps.tile([128, 512], F32, tag="fl", name="ps_v")
                g, sub = divmod(kt, NCG)
                for c in range(NHC):
                    nc.tensor.matmul(
                        pv[:, 0:FPC],
                        hs_cg[c][g][:, sub * 128 : (sub + 1) * 128],
                        w_sb["v"][c][:],
                        start=(c == 0),
                        stop=(c == NHC - 1),
                    )
                v_view = v_sb[kt][:].rearrange("p (h w) -> p h w", h=HPC)
                nc.vector.tensor_copy(
                    v_view[:, :, 0:HD],
                    pv[:, 0:FPC].rearrange("p (h w) -> p h w", h=HPC),
                )
                nc.vector.tensor_copy(v_view[:, :, HD : HD + 1].squeeze(), vones_f[:])

            # ---- pre-B: per hs column-group, q/k-hp0 + k-half1 + v ---------
            # (scores scan k over the FULL key length every unit, so k-hp0
            # must be complete before phase B; q stays lazy per half)
            psA = sc_ps.tile([128, 1024], F32, tag="sc", name="ps_q0a")
            psB = sc_ps.tile([128, 1024], F32, tag="sc", name="ps_k0a")
            psK = None
            for g in range(NCG):
                if g < 2:
                    for c in range(NHC):
                        nc.tensor.matmul(
                            psA[:, g * 512 : (g + 1) * 512],
                            w_sb["q0"][c][:],
                            hs_cg[c][g][:],
                            start=(c == 0),
                            stop=(c == NHC - 1),
                        )
                    for c in range(NHC):
                        nc.tensor.matmul(
                            psB[:, g * 512 : (g + 1) * 512],
                            w_sb["k0"][c][:],
                            hs_cg[c][g][:],
                            start=(c == 0),
                            stop=(c == NHC - 1),
                        )
                else:
                    if psK is None:
                        psK = sc_ps.tile([128, 1024], F32, tag="sc", name="ps_k0b")
                    for c in range(NHC):
                        nc.tensor.matmul(
                            psK[:, (g - 2) * 512 : (g - 1) * 512],
                            w_sb["k0"][c][:],
                            hs_cg[c][g][:],
                            start=(c == 0),
                            stop=(c == NHC - 1),
                        )
                for kt in range(g * 4, g * 4 + 4):
                    emit_v_group(kt)
                if g == 1:
                    nc.vector.tensor_scalar_add(q_sb[0][:, 0:1024], psA[:], bq_sb[:, 0:1])
                    nc.vector.tensor_scalar_add(k_sb[0][:, 0:1024], psB[:], bk_sb[:, 0:1])
            nc.vector.tensor_scalar_add(k_sb[0][:, 1024:2048], psK[:], bk_sb[:, 0:1])

            # ---- side-work closures (emitted into phase-B PE slack) --------
            # Each closure emits one PE matmul (or one DVE drain). Groups:
            # q-hp0 second half, q/k hp1 -> 10 accumulation groups of
            # [128, 512] in the floater psum + a drain each. qk_q drains
            # 2/iter in units 0-3; barriers before units that read the
            # projected regions flush any stragglers.
            qk_q = []
            op_q = []
            tail_mode = [False]

            def make_qk_group(wname, colofs, dst, bias, bcol):
                state = {}

                def mk(c, colofs=colofs, wname=wname, state=state):
                    def emit():
                        if c == 0:
                            state["ps"] = fl_ps.tile(
                                [128, 512], F32, tag="fl", name=f"ps_{wname}{colofs}"
                            )
                        nc.tensor.matmul(
                            state["ps"][:],
                            w_sb[wname][c][:],
                            hs_cg[c][colofs // 512][:],
                            start=(c == 0),
                            stop=(c == NHC - 1),
                        )

                    return emit

                for c in range(NHC):
                    qk_q.append(mk(c))

                def drain(dst=dst, colofs=colofs, bias=bias, bcol=bcol, state=state):
                    nc.vector.tensor_scalar_add(
                        dst[:, colofs : colofs + 512],
                        state["ps"][:],
                        bias[:, bcol : bcol + 1],
                    )

                qk_q.append(drain)

            for colofs in (1024, 1536):
                make_qk_group("q0", colofs, q_sb[0], bq_sb, 0)
            n_qk_half1 = len(qk_q)  # unit 1 (h0, half1) needs these done
            for colofs in (0, 512, 1024, 1536):
                make_qk_group("k1", colofs, k_sb[1], bk_sb, 1)
                make_qk_group("q1", colofs, q_sb[1], bq_sb, 1)
            n_qk_total = len(qk_q)
            qk_done = [0]

            def pop_qk(n):
                while n > 0 and qk_q:
                    qk_q.pop(0)()
                    qk_done[0] += 1
                    n -= 1

            def flush_qk_until(target):
                pop_qk(target - qk_done[0])

            # Output projection closures for one q-half: 16 groups of
            # (2 accum MMs -> drain -> DMA out). In tail mode drains
            # alternate DVE/ACT and DMAs alternate sync/scalar.
            def push_oproj_half(half):
                for lt in range(half * 8, half * 8 + 8):
                    for nch in range(2):
                        state = {}
                        gi = lt * 2 + nch

                        def mm(cc, lt=lt, nch=nch, state=state):
                            def emit():
                                if cc == 0:
                                    state["ps"] = fl_ps.tile(
                                        [128, 512], F32, tag="fl", name="ps_o"
                                    )
                                nc.tensor.matmul(
                                    state["ps"][:],
                                    ctx_sb[cc][:, lt * 128 : (lt + 1) * 128],
                                    wo_sb[cc][:, nch * 512 : (nch + 1) * 512],
                                    start=(cc == 0),
                                    stop=(cc == 1),
                                )

                            return emit

                        op_q.append(mm(0))
                        op_q.append(mm(1))

                        def drain(lt=lt, nch=nch, gi=gi, state=state):
                            o_sb = work.tile(
                                [128, 512], BF16, tag="ostage", name="o_sb", bufs=4
                            )
                            if tail_mode[0] and gi % 2:
                                nc.scalar.activation(
                                    o_sb[:],
                                    state["ps"][:],
                                    mybir.ActivationFunctionType.Copy,
                                )
                                eng = nc.scalar
                            else:
                                nc.vector.tensor_copy(o_sb[:], state["ps"][:])
                                eng = nc.sync
                            eng.dma_start(
                                out=out[
                                    lt * 128 : (lt + 1) * 128,
                                    nch * 512 : (nch + 1) * 512,
                                ],
                                in_=o_sb[:],
                            )

                        op_q.append(drain)

            # ---- phase B ---------------------------------------------------
            fill_state = {"ps": None, "n": 0}

            def close_filler(h):
                if fill_state["ps"] is not None:
                    nc.tensor.matmul(
                        fill_state["ps"][:],
                        v_sb[0][:, h * 65 : (h + 1) * 65],
                        hs_cg[0][0][:],
                        start=False,
                        stop=True,
                        skip_group_check=True,
                    )
                    fcopy = work.tile([65, 512], F32, tag="fguard", name="fcopy", bufs=2)
                    nc.vector.tensor_copy(fcopy[:], fill_state["ps"][:])
                    nc.sync.dma_start(out=scratch[0:65, :], in_=fcopy[:])
                    fill_state["ps"] = None

            def normalize(h, half, cg):
                hp, hr = divmod(h, 2)
                raw = work.tile([65, 1024], F32, tag="raw", name="raw", bufs=2)
                for g2 in range(2):
                    nc.vector.tensor_copy(raw[:, g2 * 512 : (g2 + 1) * 512], cg[g2][:])
                d_dram = dscratch.tile([1, 1024], F32, tag="ddram", name="d_dram")
                nc.sync.dma_start(out=d_dram[:], in_=raw[64:65, :])
                dbc = work.tile([64, 1024], F32, tag="dbc", name="dbc", bufs=2)
                nc.sync.dma_start(
                    out=dbc[:], in_=d_dram[0:1, :].to_broadcast([64, 1024])
                )
                rbc = work.tile([64, 1024], F32, tag="rbc", name="rbc", bufs=2)
                nc.vector.reciprocal_approx_fast(rbc[:], dbc[:])
                nc.vector.tensor_mul(
                    ctx_sb[hp][hr * HD : (hr + 1) * HD, half * 1024 : (half + 1) * 1024],
                    raw[0:64, :],
                    rbc[:],
                )

            for h in range(HPC):
                hp, hr = divmod(h, 2)
                q_head = q_sb[hp][hr * HD : (hr + 1) * HD, :]
                k_head = k_sb[hp][hr * HD : (hr + 1) * HD, :]
                for half in range(2):
                    unit = h * 2 + half
                    qoff = half * 1024
                    # correctness barriers: the unit's q/k regions must be
                    # fully projected before its scores MMs are emitted
                    if unit == 1:
                        flush_qk_until(n_qk_half1)
                    elif unit == 4:
                        flush_qk_until(n_qk_total)
                    cg = [
                        cg_ps.tile([65, 512], F32, tag="cg", name=f"cg{h}{half}{g2}")
                        for g2 in range(2)
                    ]
                    prev = None

                    def emit_ctx(prev, h=h, cg=cg):
                        kt0, e = prev
                        for g2 in range(2):
                            nc.tensor.matmul(
                                cg[g2][:],
                                v_sb[kt0][:, h * 65 : (h + 1) * 65],
                                e[:, g2 * 512 : (g2 + 1) * 512],
                                start=(kt0 == 0),
                                stop=(kt0 == NKT - 1),
                            )

                    for kt in range(NKT):
                        psS = sc_ps.tile([128, 1024], F32, tag="sc", name="ps_s")
                        for s2 in range(2):
                            nc.tensor.matmul(
                                psS[:, s2 * 512 : (s2 + 1) * 512],
                                k_head[:, kt * 128 : (kt + 1) * 128],
                                q_head[:, qoff + s2 * 512 : qoff + (s2 + 1) * 512],
                                start=True,
                                stop=True,
                            )
                        # side work in the PE slack of the ACT-paced loop
                        if qk_q:
                            pop_qk(2)
                        elif op_q:
                            for _ in range(3):
                                if op_q:
                                    op_q.pop(0)()
                        elif 2 <= unit <= 6:
                            # keepalive: HAM ignores K=64 scores; without
                            # K=128 side work the PE clock pins at 1.2 GHz
                            if fill_state["ps"] is None:
                                fill_state["ps"] = fl_ps.tile(
                                    [65, 512], F32, tag="fl", name="fill_ps"
                                )
                                fill_state["n"] = 0
                            nc.tensor.matmul(
                                fill_state["ps"][:],
                                v_sb[kt][:, h * 65 : (h + 1) * 65],
                                hs_cg[0][0][:],
                                start=(fill_state["n"] == 0),
                                stop=False,
                                skip_group_check=True,
                            )
                            fill_state["n"] += 1
                        if prev is not None:
                            emit_ctx(prev)
                        e_t = work.tile([128, 1024], BF16, tag="e", name="e_t", bufs=4)
                        nc.scalar.activation(
                            e_t[:],
                            psS[:],
                            mybir.ActivationFunctionType.Exp,
                            bias=del8_sb[:, kt : kt + 1],
                            scale=tau_sb[:],
                        )
                        prev = (kt, e_t)
                    emit_ctx(prev)
                    close_filler(h)
                    normalize(h, half, cg)
                    if h == HPC - 1 and half == 0:
                        push_oproj_half(0)

            tail_mode[0] = True
            push_oproj_half(1)
            while op_q:
                op_q.pop(0)()

    nc.compile()
    return nc


def _get_nc():
    if "nc" not in _NC_CACHE:
        _NC_CACHE["nc"] = _build_kernel()
    return _NC_CACHE["nc"]


def _make_in_maps(hidden_states, tau, delta, Wq, Wk, Wv, Wo, bq, bk):
    def bfc(x):
        return np.ascontiguousarray(x).astype(NPBF)

    in_maps = []
    for c in range(NCORES):
        b, hg = divmod(c, HPC)
        fs = slice(hg * FPC, (hg + 1) * FPC)
        wq_t = Wq[fs, :].T  # [H, FPC]
        wk_t = Wk[fs, :].T
        in_maps.append(
            {
                "hs_t": bfc(hidden_states[b].T),
                "wq0": bfc(wq_t[:, 0:128]),
                "wq1": bfc(wq_t[:, 128:256]),
                "wk0": bfc(wk_t[:, 0:128]),
                "wk1": bfc(wk_t[:, 128:256]),
                "wv_t": bfc(Wv[fs, :].T),
                "wo_t": bfc(Wo[:, fs].T),
                "bq2": np.ascontiguousarray(bq[fs].reshape(2, 128).T),
                "bk2": np.ascontiguousarray(bk[fs].reshape(2, 128).T),
                "tau8": np.full((128, 1), tau[b, 0] / 8.0, dtype=np.float32),
                "delta8": np.ascontiguousarray((delta[b] / 8.0).reshape(NKT, 128).T),
            }
        )
    return in_maps


def kernel(hidden_states, tau, delta, Wq, bq, Wk, bk, Wv, bv, Wo, bo, _trace=False):
    hidden_states = np.asarray(hidden_states, dtype=np.float32)
    tau = np.asarray(tau, dtype=np.float32)
    delta = np.asarray(delta, dtype=np.float32)
    Wq = np.asarray(Wq, dtype=np.float32)
    Wk = np.asarray(Wk, dtype=np.float32)
    Wv = np.asarray(Wv, dtype=np.float32)
    Wo = np.asarray(Wo, dtype=np.float32)
    bq = np.asarray(bq, dtype=np.float32)
    bk = np.asarray(bk, dtype=np.float32)
    bv = np.asarray(bv, dtype=np.float32)
    bo = np.asarray(bo, dtype=np.float32)

    nc = _get_nc()
    in_maps = _make_in_maps(hidden_states, tau, delta, Wq, Wk, Wv, Wo, bq, bk)
    res = run_bass_kernel_spmd(nc, in_maps, list(range(NCORES)), trace=_trace)

    out = np.zeros((B, L, H), dtype=np.float32)
    for c in range(NCORES):
        out[c // HPC] += res.results[c]["out"].astype(np.float32)
    # v/out-proj biases commute through softmax-normalized attention exactly
    out += bv @ Wo.T + bo
    if _trace:
        kernel._last_exec_time_ns = res.exec_time_ns
        kernel._last_profile_json = res.profile_json
    return out
